# revision 10
# baseline (speedup 1.0000x reference)
"""Trainium2 Bass kernel for nn_MHA_2688649527670.

Reference computes, per batch b and head h:
    Q = x Wq_h^T, K = x Wk_h^T, V = x Wv_h^T          ([S, D] each)
    Z = softmax_over_d( (Q K^T / sqrt(D)) V )

No softmax between Q K^T and V, so the chain is associative:
    (Q K^T) V = x (Wq_h^T Wk_h G Wv_h^T) / sqrt(D),   G = x^T x   ([D, D])

which collapses the O(S^2 D) attention into a [D,D] weight chain plus one
[S,D]x[D,D*H] matmul, then softmax over d (free axis). Per-head softmax bias
is mandatory: per-head/row logit ranges span thousands.

Sharding: batch (4) x head-groups (2x4 heads) = 8 independent cores.

Perf notes (v4, HW-measured):
  - x loaded ROW-BLOCK: partition p holds rows 16p..16p+15, so each DMA
    descriptor is 2KB contiguous; 4 dma_starts land x ~6us earlier than the
    512B-line layout. G is chunk-order invariant (chunk n = rows {16p+n});
    host reorders the output (s = 16p + n).
  - G emitted as 4 accumulation sub-chains (start=False continuation) so the
    scheduler can interleave xT transposes / P0T between groups.
  - chain-critical copies (g, p0t, ut, m) run under tc.high_priority().
  - finals/UT in float32r (1 cyc/row at N=512); f32r written by producer.
  - epilogue per chunk (baseline-proven AP shapes, EXP=258ns needs 2D y
    and per-chunk tiles): V reduce_max -> 4x scalar Exp (per-head bias) ->
    V reduce_sum -> V reciprocal -> gpsimd normalize-mult (bf16 out) ->
    contiguous bf16 DMA out on the sync queue (host reorders/upcasts).
"""

import ml_dtypes
import numpy as np

import concourse.bass as bass
import concourse.bacc as bacc
import concourse.mybir as mybir
import concourse.tile as tile
from concourse.bass_utils import run_bass_kernel_spmd
from concourse.masks import make_identity

B, S, D, H = 4, 2048, 128, 8
P = 128
HPC = H // 2          # heads per core
NCH = S // P          # 16 chunks; chunk n = rows {16p + n}
N_CORES = 8
SCALE = 1.0 / float(np.sqrt(D))
F32 = mybir.dt.float32
F32R = mybir.dt.float32r
BF16 = mybir.dt.bfloat16

N_WARM = 4

_PROG = None


def _build_program():
    nc = bacc.Bacc("TRN2", target_bir_lowering=False, debug=False,
                   num_devices=N_CORES)

    x_d = nc.dram_tensor("x", [S, D], F32, kind="ExternalInput")
    wq_d = nc.dram_tensor("wq", [HPC * D, D], F32, kind="ExternalInput")
    wk_d = nc.dram_tensor("wk", [HPC * D, D], F32, kind="ExternalInput")
    wv_d = nc.dram_tensor("wv", [HPC * D, D], F32, kind="ExternalInput")
    # [chunk, p, head, d] bf16; row s = 16p + chunk; host reorders
    out_d = nc.dram_tensor("out", [NCH, P, HPC, D], BF16, kind="ExternalOutput")

    with tile.TileContext(nc) as tc:
        with (
            tc.tile_pool(name="const", bufs=1) as const,
            tc.tile_pool(name="work", bufs=6) as work,
            tc.tile_pool(name="small", bufs=4) as small,
            tc.tile_pool(name="ps_y", bufs=4, space="PSUM") as ps_y,
            tc.tile_pool(name="ps_g", bufs=1, space="PSUM") as ps_g,
            tc.tile_pool(name="ps_t", bufs=2, space="PSUM") as ps_t,
        ):
            ident = const.tile([P, P], F32, tag="ident")
            make_identity(nc, ident)

            # ---- input DMAs: x row-block on the two HW queues, weights on
            #      gpsimd SWDGE ----
            x_sb = const.tile([P, NCH, D], F32, tag="x_sb")
            x_view = x_d.ap().rearrange("(p n) c -> p n c", p=P)
            for q in range(4):
                eng = nc.sync if q % 2 == 0 else nc.scalar
                eng.dma_start(x_sb[:, 4 * q:4 * q + 4, :],
                              x_view[:, 4 * q:4 * q + 4, :])
            w_sb = {}
            for nm, wd in (("wq", wq_d), ("wk", wk_d), ("wv", wv_d)):
                t = const.tile([P, HPC, D], F32, tag=f"{nm}_sb", name=f"{nm}_sb")
                nc.gpsimd.dma_start(t, wd.ap().rearrange("(h p) c -> p h c", p=P))
                w_sb[nm] = t

            # ---- PE p-state warmup matmuls on a memset tile ----
            warm = const.tile([P, P], F32, tag="warm")
            nc.gpsimd.memset(warm, 0.0)
            g_ps = ps_g.tile([P, P], F32, tag="g_ps")
            for _ in range(N_WARM):
                nc.tensor.matmul(g_ps, lhsT=warm, rhs=warm, start=True,
                                 stop=True)

            # ---- P0T / WvT first: weights land (~10us) before x (~11.5us),
            #      so these ride the DMA wait ahead of G ----
            p0t_ps = ps_y.tile([P, HPC * D], F32, tag="c_ps")
            p0t_sb = const.tile([P, HPC * D], F32R, tag="p0t_sb")
            wvt_ps = ps_y.tile([P, HPC * D], F32, tag="c_ps")
            wvt_sb = const.tile([P, HPC * D], F32R, tag="wvt_sb")
            for h in range(HPC):
                nc.tensor.matmul(p0t_ps[:, h * D:(h + 1) * D],
                                 lhsT=w_sb["wk"][:, h, :],
                                 rhs=w_sb["wq"][:, h, :])
            nc.scalar.mul(p0t_sb, p0t_ps, SCALE)
            for h in range(HPC):
                nc.tensor.transpose(wvt_ps[:, h * D:(h + 1) * D],
                                    w_sb["wv"][:, h, :], ident)
            nc.scalar.copy(wvt_sb, wvt_ps)

            # ---- G = x^T x as 4 quarter-aligned sub-chains (pipeline under
            #      the x DMA) ----
            for grp in range(4):
                for i in range(4 * grp, 4 * grp + 4):
                    nc.tensor.matmul(g_ps, lhsT=x_sb[:, i, :],
                                     rhs=x_sb[:, i, :],
                                     start=(i == 0), stop=(i == NCH - 1),
                                     skip_group_check=(i != 0 and i % 4 == 0))

            g_sb = const.tile([P, P], F32R, tag="g_sb")
            with tc.high_priority():
                nc.vector.tensor_copy(g_sb, g_ps)

            # ---- xT transposes: just-in-time, woven into the finals loop ----
            xT_sb = const.tile([P, NCH, D], F32R, tag="xT_sb")
            tp_banks = {}

            def emit_xt(i):
                b = i // 4
                if i % 4 == 0:
                    tp_banks[b] = ps_t.tile([P, 4, P], F32, tag="tp",
                                            name=f"tp{b}")
                nc.tensor.transpose(tp_banks[b][:, i % 4, :],
                                    x_sb[:, i, :], ident)

            def emit_cast2(j):  # chunks 2j, 2j+1
                b = j // 2
                src = tp_banks[b][:, 2 * (j % 2):2 * (j % 2) + 2, :]
                dst = xT_sb[:, 2 * j:2 * j + 2, :]
                if j % 2 == 0:
                    nc.vector.tensor_copy(dst, src)
                else:
                    nc.scalar.copy(dst, src)

            # ---- UT = G @ P0T (G symmetric), one N=512 f32r matmul ----
            ut_ps = ps_y.tile([P, HPC * D], F32, tag="c_ps")
            ut_sb = const.tile([P, HPC * D], F32R, tag="ut_sb")
            m_ps = ps_y.tile([P, HPC * D], F32, tag="c_ps")
            m_all = const.tile([P, HPC * D], F32R, tag="m_all")
            HW = HPC * D // 2
            with tc.high_priority():
                nc.tensor.matmul(ut_ps, lhsT=g_sb, rhs=p0t_sb)
                nc.vector.tensor_copy(ut_sb[:, :HW], ut_ps[:, :HW])
                nc.scalar.copy(ut_sb[:, HW:], ut_ps[:, HW:])
                for h in range(HPC):
                    sl = slice(h * D, (h + 1) * D)
                    nc.tensor.matmul(m_ps[:, sl], lhsT=ut_sb[:, sl],
                                     rhs=wvt_sb[:, sl])
                nc.vector.tensor_copy(m_all[:, :HW], m_ps[:, :HW])
                nc.scalar.copy(m_all[:, HW:], m_ps[:, HW:])

            # ---- finals + software-pipelined softmax epilogue ----
            t_live = {}

            def emit_front(i):
                y_ps = ps_y.tile([P, HPC * D], F32, tag="c_ps")
                nc.tensor.matmul(y_ps, lhsT=xT_sb[:, i, :], rhs=m_all[:])
                negmax = small.tile([P, HPC], F32, tag="negmax")
                nc.vector.reduce_max(
                    out=negmax,
                    in_=y_ps[:].rearrange("p (h d) -> p h d", h=HPC),
                    axis=mybir.AxisListType.X, negate=True)
                t_sb = work.tile([P, HPC, D], F32, tag="t_sb")
                for h in range(HPC):
                    nc.scalar.activation(
                        t_sb[:, h, :], y_ps[:, h * D:(h + 1) * D],
                        mybir.ActivationFunctionType.Exp,
                        bias=negmax[:, h:h + 1], scale=1.0)
                t_live[i] = t_sb

            def emit_back(i):
                t_sb = t_live.pop(i)
                sums = small.tile([P, HPC], F32, tag="sums")
                nc.vector.reduce_sum(out=sums, in_=t_sb,
                                     axis=mybir.AxisListType.X)
                rsum = small.tile([P, HPC], F32, tag="rsum")
                nc.vector.reciprocal(rsum, sums)
                o_sb = work.tile([P, HPC, D], BF16, tag="o_sb")
                nc.gpsimd.tensor_tensor(
                    o_sb, t_sb,
                    rsum[:, :, None].to_broadcast((P, HPC, D)),
                    mybir.AluOpType.mult)
                nc.sync.dma_start(out_d.ap()[i], o_sb)

            def emit_xt2(j):  # transposes + cast for chunks 2j, 2j+1
                emit_xt(2 * j)
                emit_xt(2 * j + 1)
                emit_cast2(j)

            emit_xt2(0)
            emit_xt2(1)
            emit_front(0)
            for i in range(1, NCH):
                if i % 2 == 1 and i < NCH - 3:
                    emit_xt2((i + 3) // 2)
                emit_front(i)
                emit_back(i - 1)
            emit_back(NCH - 1)

    nc.compile()
    return nc


def _get_program():
    global _PROG
    if _PROG is None:
        _PROG = _build_program()
    return _PROG


def _make_in_maps(x, W_q, W_k, W_v):
    in_maps = []
    for core in range(N_CORES):
        b, hg = core // 2, core % 2
        sl = slice(hg * HPC * D, (hg + 1) * HPC * D)
        in_maps.append({
            "x": np.ascontiguousarray(x[b]),
            "wq": np.ascontiguousarray(W_q[sl]),
            "wk": np.ascontiguousarray(W_k[sl]),
            "wv": np.ascontiguousarray(W_v[sl]),
        })
    return in_maps


def run(x, W_q, W_k, W_v, trace=False, **spmd_kwargs):
    """Run on 8 NeuronCores; returns (Z, BassKernelResults)."""
    nc = _get_program()
    in_maps = _make_in_maps(np.asarray(x, np.float32), np.asarray(W_q, np.float32),
                            np.asarray(W_k, np.float32), np.asarray(W_v, np.float32))
    res = run_bass_kernel_spmd(nc, in_maps, core_ids=list(range(N_CORES)),
                               trace=trace, **spmd_kwargs)
    Z = np.empty((B, H, S, D), np.float32)
    for core in range(N_CORES):
        b, hg = core // 2, core % 2
        o = np.asarray(res.results[core]["out"]).astype(np.float32)
        # [chunk n, p, h, d] -> [h, s=16p+n, d]
        Z[b, hg * HPC:(hg + 1) * HPC] = (
            o.transpose(2, 1, 0, 3).reshape(HPC, S, D))
    return Z, res


def kernel(x, W_q, W_k, W_v):
    Z, _ = run(x, W_q, W_k, W_v, trace=False)
    return Z


# revision 12
# speedup vs baseline: 1.0455x; 1.0455x over previous
"""Trainium2 Bass kernel for nn_MHA_2688649527670.

Reference computes, per batch b and head h:
    Q = x Wq_h^T, K = x Wk_h^T, V = x Wv_h^T          ([S, D] each)
    Z = softmax_over_d( (Q K^T / sqrt(D)) V )

No softmax between Q K^T and V, so the chain is associative:
    (Q K^T) V = x (Wq_h^T Wk_h G Wv_h^T) / sqrt(D),   G = x^T x   ([D, D])

which collapses the O(S^2 D) attention into a [D,D] weight chain plus one
[S,D]x[D,D*H] matmul, then softmax over d (free axis). Per-head softmax bias
is mandatory: per-head/row logit ranges span thousands.

Sharding: batch (4) x head-groups (2x4 heads) = 8 independent cores.

Perf notes (v4, HW-measured):
  - x loaded ROW-BLOCK: partition p holds rows 16p..16p+15, so each DMA
    descriptor is 2KB contiguous; 4 dma_starts land x ~6us earlier than the
    512B-line layout. G is chunk-order invariant (chunk n = rows {16p+n});
    host reorders the output (s = 16p + n).
  - G emitted as 4 accumulation sub-chains (start=False continuation) so the
    scheduler can interleave xT transposes / P0T between groups.
  - chain-critical copies (g, p0t, ut, m) run under tc.high_priority().
  - finals/UT in float32r (1 cyc/row at N=512); f32r written by producer.
  - epilogue per chunk (baseline-proven AP shapes, EXP=258ns needs 2D y
    and per-chunk tiles): V reduce_max -> 4x scalar Exp (per-head bias) ->
    V reduce_sum -> V reciprocal -> gpsimd normalize-mult (bf16 out) ->
    contiguous bf16 DMA out on the sync queue (host reorders/upcasts).
"""

import ml_dtypes
import numpy as np

import concourse.bass as bass
import concourse.bacc as bacc
import concourse.mybir as mybir
import concourse.tile as tile
from concourse.bass_utils import run_bass_kernel_spmd
from concourse.masks import make_identity

B, S, D, H = 4, 2048, 128, 8
P = 128
HPC = H // 2          # heads per core
NCH = S // P          # 16 chunks; chunk n = rows {16p + n}
N_CORES = 8
SCALE = 1.0 / float(np.sqrt(D))
F32 = mybir.dt.float32
F32R = mybir.dt.float32r
BF16 = mybir.dt.bfloat16

N_WARM = 2

_PROG = None


def _build_program():
    nc = bacc.Bacc("TRN2", target_bir_lowering=False, debug=False,
                   num_devices=N_CORES)

    x_d = nc.dram_tensor("x", [S, D], F32, kind="ExternalInput")
    wq_d = nc.dram_tensor("wq", [HPC * D, D], F32, kind="ExternalInput")
    wk_d = nc.dram_tensor("wk", [HPC * D, D], F32, kind="ExternalInput")
    # wv arrives HOST-TRANSPOSED: [d, (h e)] so WvT needs no PE transposes
    wv_d = nc.dram_tensor("wv", [D, HPC * D], F32, kind="ExternalInput")
    # [chunk, p, head, d] bf16; row s = 16p + chunk; host reorders
    out_d = nc.dram_tensor("out", [NCH, P, HPC, D], BF16, kind="ExternalOutput")

    with tile.TileContext(nc) as tc:
        with (
            tc.tile_pool(name="const", bufs=1) as const,
            tc.tile_pool(name="work", bufs=6) as work,
            tc.tile_pool(name="small", bufs=4) as small,
            tc.tile_pool(name="ps_y", bufs=4, space="PSUM") as ps_y,
            tc.tile_pool(name="ps_g", bufs=1, space="PSUM") as ps_g,
            tc.tile_pool(name="ps_t", bufs=2, space="PSUM") as ps_t,
            tc.tile_pool(name="ps_gb", bufs=1, space="PSUM") as ps_gb,
        ):
            ident = const.tile([P, P], F32, tag="ident")
            make_identity(nc, ident)

            # ---- input DMAs: x row-block on the two HW queues, weights on
            #      gpsimd SWDGE ----
            x_sb = const.tile([P, NCH, D], F32, tag="x_sb")
            x_view = x_d.ap().rearrange("(p n) c -> p n c", p=P)
            for q in range(4):
                eng = nc.sync if q % 2 == 0 else nc.scalar
                eng.dma_start(x_sb[:, 4 * q:4 * q + 4, :],
                              x_view[:, 4 * q:4 * q + 4, :])
            w_sb = {}
            for nm, wd in (("wq", wq_d), ("wk", wk_d)):
                t = const.tile([P, HPC, D], F32, tag=f"{nm}_sb", name=f"{nm}_sb")
                nc.gpsimd.dma_start(t, wd.ap().rearrange("(h p) c -> p h c", p=P))
                w_sb[nm] = t
            wvt_sb = const.tile([P, HPC * D], F32, tag="wvt_sb")
            nc.gpsimd.dma_start(wvt_sb, wv_d.ap())

            # ---- PE p-state warmup matmuls on a memset tile ----
            warm = const.tile([P, P], F32, tag="warm")
            nc.gpsimd.memset(warm, 0.0)
            g_ps = ps_g.tile([P, P], F32, tag="g_ps")
            for _ in range(N_WARM):
                nc.tensor.matmul(g_ps, lhsT=warm, rhs=warm, start=True,
                                 stop=True)

            # ---- G = x^T x in two halves (UT accumulates per half), with
            #      xT transposes and P0T interleaved between quarter groups ----
            xT_sb = const.tile([P, NCH, D], F32R, tag="xT_sb")
            tp_banks = {}

            def emit_xt(i):
                b = i // 4
                if i % 4 == 0:
                    tp_banks[b] = ps_t.tile([P, 4, P], F32, tag="tp",
                                            name=f"tp{b}")
                nc.tensor.transpose(tp_banks[b][:, i % 4, :],
                                    x_sb[:, i, :], ident)

            def emit_cast(b):
                if b % 2 == 0:
                    nc.vector.tensor_copy(xT_sb[:, 4 * b:4 * b + 4, :],
                                          tp_banks[b])
                else:
                    nc.scalar.copy(xT_sb[:, 4 * b:4 * b + 4, :], tp_banks[b])

            p0t_ps = ps_y.tile([P, HPC * D], F32, tag="c_ps")
            p0t_sb = const.tile([P, HPC * D], F32R, tag="p0t_sb")
            g_a = const.tile([P, P], F32R, tag="g_a")
            g_b = const.tile([P, P], F32R, tag="g_b")
            gb_ps = ps_gb.tile([P, P], F32, tag="g_b_ps")

            for grp in range(4):
                half = gb_ps if grp >= 2 else g_ps
                for i in range(4 * grp, 4 * grp + 4):
                    nc.tensor.matmul(half, lhsT=x_sb[:, i, :],
                                     rhs=x_sb[:, i, :],
                                     start=(i % 8 == 0), stop=(i % 8 == 7),
                                     skip_group_check=(i % 8 == 4))
                for i in range(4 * grp, 4 * grp + 4):
                    emit_xt(i)
                if grp == 1:
                    with tc.high_priority():
                        nc.vector.tensor_copy(g_a, g_ps)
                    emit_cast(0)
                if grp == 2:
                    for h in range(HPC):
                        nc.tensor.matmul(p0t_ps[:, h * D:(h + 1) * D],
                                         lhsT=w_sb["wk"][:, h, :],
                                         rhs=w_sb["wq"][:, h, :])
                    nc.scalar.mul(p0t_sb, p0t_ps, SCALE)
                    emit_cast(1)
            with tc.high_priority():
                nc.vector.tensor_copy(g_b, gb_ps)
            emit_cast(2)
            emit_cast(3)

            # ---- UT = (G_a + G_b) @ P0T via accumulation; M per head ----
            ut_ps = ps_y.tile([P, HPC * D], F32, tag="c_ps")
            ut_sb = const.tile([P, HPC * D], F32, tag="ut_sb")
            m_ps = ps_y.tile([P, HPC * D], F32, tag="c_ps")
            m_all = const.tile([P, HPC * D], F32R, tag="m_all")
            HW = HPC * D // 2
            with tc.high_priority():
                nc.tensor.matmul(ut_ps, lhsT=g_a, rhs=p0t_sb,
                                 start=True, stop=False)
                nc.tensor.matmul(ut_ps, lhsT=g_b, rhs=p0t_sb,
                                 start=False, stop=True)
                nc.vector.tensor_copy(ut_sb[:, :HW], ut_ps[:, :HW])
                nc.scalar.copy(ut_sb[:, HW:], ut_ps[:, HW:])
                for h in range(HPC):
                    sl = slice(h * D, (h + 1) * D)
                    nc.tensor.matmul(m_ps[:, sl], lhsT=ut_sb[:, sl],
                                     rhs=wvt_sb[:, sl])
                    if h % 2 == 0:
                        nc.vector.tensor_copy(m_all[:, sl], m_ps[:, sl])
                    else:
                        nc.scalar.copy(m_all[:, sl], m_ps[:, sl])

            # ---- finals + software-pipelined softmax epilogue ----
            t_live = {}

            def emit_front(i):
                y_ps = ps_y.tile([P, HPC * D], F32, tag="c_ps")
                nc.tensor.matmul(y_ps, lhsT=xT_sb[:, i, :], rhs=m_all[:])
                negmax = small.tile([P, HPC], F32, tag="negmax")
                nc.vector.reduce_max(
                    out=negmax,
                    in_=y_ps[:].rearrange("p (h d) -> p h d", h=HPC),
                    axis=mybir.AxisListType.X, negate=True)
                t_sb = work.tile([P, HPC, D], F32, tag="t_sb")
                for h in range(HPC):
                    nc.scalar.activation(
                        t_sb[:, h, :], y_ps[:, h * D:(h + 1) * D],
                        mybir.ActivationFunctionType.Exp,
                        bias=negmax[:, h:h + 1], scale=1.0)
                t_live[i] = t_sb

            def emit_back(i):
                t_sb = t_live.pop(i)
                sums = small.tile([P, HPC], F32, tag="sums")
                nc.vector.reduce_sum(out=sums, in_=t_sb,
                                     axis=mybir.AxisListType.X)
                rsum = small.tile([P, HPC], F32, tag="rsum")
                nc.vector.reciprocal(rsum, sums)
                o_sb = work.tile([P, HPC, D], BF16, tag="o_sb")
                nc.gpsimd.tensor_tensor(
                    o_sb, t_sb,
                    rsum[:, :, None].to_broadcast((P, HPC, D)),
                    mybir.AluOpType.mult)
                nc.sync.dma_start(out_d.ap()[i], o_sb)

            emit_front(0)
            for i in range(1, NCH):
                emit_front(i)
                emit_back(i - 1)
            emit_back(NCH - 1)

    nc.compile()
    return nc


def _get_program():
    global _PROG
    if _PROG is None:
        _PROG = _build_program()
    return _PROG


def _make_in_maps(x, W_q, W_k, W_v):
    in_maps = []
    for core in range(N_CORES):
        b, hg = core // 2, core % 2
        sl = slice(hg * HPC * D, (hg + 1) * HPC * D)
        in_maps.append({
            "x": np.ascontiguousarray(x[b]),
            "wq": np.ascontiguousarray(W_q[sl]),
            "wk": np.ascontiguousarray(W_k[sl]),
            "wv": np.ascontiguousarray(W_v[sl].T),
        })
    return in_maps


def run(x, W_q, W_k, W_v, trace=False, **spmd_kwargs):
    """Run on 8 NeuronCores; returns (Z, BassKernelResults)."""
    nc = _get_program()
    in_maps = _make_in_maps(np.asarray(x, np.float32), np.asarray(W_q, np.float32),
                            np.asarray(W_k, np.float32), np.asarray(W_v, np.float32))
    res = run_bass_kernel_spmd(nc, in_maps, core_ids=list(range(N_CORES)),
                               trace=trace, **spmd_kwargs)
    Z = np.empty((B, H, S, D), np.float32)
    for core in range(N_CORES):
        b, hg = core // 2, core % 2
        o = np.asarray(res.results[core]["out"]).astype(np.float32)
        # [chunk n, p, h, d] -> [h, s=16p+n, d]
        Z[b, hg * HPC:(hg + 1) * HPC] = (
            o.transpose(2, 1, 0, 3).reshape(HPC, S, D))
    return Z, res


def kernel(x, W_q, W_k, W_v):
    Z, _ = run(x, W_q, W_k, W_v, trace=False)
    return Z
